# revision 1
# baseline (speedup 1.0000x reference)
"""Trainium2 Bass kernel for nn_DensityDecoder (gnn_message_passing).

Math: the reference computes, for every ordered pair (i, j) of NB=640 orbitals,
    pair = orb_i + orb_j                       (orb: per-orbital projected embedding)
    qn   = LayerNorm(pair) ; q = qn @ Wq + bq
    attn = softmax(q . k / sqrt(Dh)) over a tiny T=32 latent KV
    out  = MLP(attn @ V @ Wo)  ->  2 values -> rho[i, j] = out0 + 1j*out1

Because pair = orb_i + orb_j, the LN statistics decompose exactly:
    mu_ij  = mu_i + mu_j
    var_ij = msq_i + msq_j + 2*G_ij - mu_ij^2        (G = orb @ orb.T / D)
and the whole pre-softmax pipeline collapses to per-orbital precomputes
(SA = ((orb*g) @ Wq) projected into (head, token) score space, plus the
constant vectors Sw, Sb), so the 410MB pair tensor is never materialized:
    scores_ij = rstd_ij * (SA_i + SA_j - mu_ij*Sw) + Sb      (pre-scaled by 1/sqrt(Dh))
The per-pair device work is the softmax + a 5-layer MLP chain, where
attn @ V @ Wo is folded into one 256x256 matmul (Wvo = blockdiag(V) @ Wo).

rho is exactly symmetric (pair_ij == pair_ji bitwise), so only j-blocks >=
i-block are computed (240 of 400 row-tiles) and the lower triangle is mirrored.

Sharding: rows i are striped across the 8 cores (i % 8 == core), giving every
core an IDENTICAL instruction stream (same NEFF, SPMD) over different data:
80 rows -> 240 tiles of 128 pairs -> 60 chain-chunks of 512 pairs.
"""

import os
import numpy as np

EPS = 1e-5
H = 8
D = 256
T = 32
Dh = D // H
NB = 640
NCORES = 8
NBLK = NB // 128          # 5 column blocks
RPB = 128 // NCORES       # 16 rows per block per core
NROWS = NBLK * RPB        # 80 rows per core
# tile enumeration (identical on every core): (block, row-in-block, j-block)
TILES = [(B, r, jt) for B in range(NBLK) for r in range(RPB) for jt in range(B, NBLK)]
NTILES = len(TILES)       # 240
CHUNK = 4                 # tiles per MLP-chain chunk (512 pair columns)
GROUP = int(os.environ.get("DD_GROUP", "8"))  # chunks per superchunk (ACT-phase granularity)
NCHUNKS = NTILES // CHUNK  # 60

_CACHE = {}


def _silu(x):
    return x / (1.0 + np.exp(-x))


def _ln(x, g, b):
    mu = x.mean(-1, keepdims=True)
    var = x.var(-1, keepdims=True)
    return (x - mu) / np.sqrt(var + EPS) * g + b


def _precompute(inputs):
    """Pair-independent precompute (all O(NB*D) or smaller), numpy float64 -> float32."""
    f = {}
    for k, v in inputs.items():
        v = np.asarray(v)
        f[k] = v.astype(np.float64) if v.dtype in (np.float32, np.float64) else v
    Z = np.asarray(inputs["Z"]).astype(np.int64)
    l = np.asarray(inputs["l"]).astype(np.int64)
    m = np.asarray(inputs["m"]).astype(np.int64)
    m_idx = np.clip(m + 3, 0, 4)
    emb = np.concatenate([f["elem_tab"][Z], f["l_tab"][l], f["m_tab"][m_idx]], -1)
    orb = _silu(emb @ f["Wp0"] + f["bp0"]) @ f["Wp1"] + f["bp1"]          # (NB, D)

    kv = _ln(f["latent"], f["ln_gkv"], f["ln_bkv"])
    k = (kv @ f["Wk"] + f["bk"]).reshape(T, H, Dh)
    v = (kv @ f["Wv"] + f["bv"]).reshape(T, H, Dh)

    g, b = f["ln_gq"], f["ln_bq"]
    mu = orb.mean(-1)
    msq = (orb * orb).mean(-1)

    A = (orb * g) @ f["Wq"]
    wbar = g @ f["Wq"]
    bq_eff = b @ f["Wq"] + f["bqa"]

    kT = k.transpose(1, 2, 0)                                            # (H, Dh, T)
    scale = 1.0 / np.sqrt(np.float64(Dh))

    def to_scores(x):
        xh = x.reshape(x.shape[:-1] + (H, Dh))
        return (np.einsum('...hd,hdt->...ht', xh, kT).reshape(x.shape[:-1] + (H * T,))
                * scale)

    SA = to_scores(A)                                                    # (NB, 256)
    Sw = to_scores(wbar)                                                 # (256,)
    Sb = to_scores(bq_eff)                                               # (256,)
    Wvo = np.einsum('thd,hde->hte', v, f["Wo"].reshape(H, Dh, D)).reshape(H * T, D)
    # fuse consecutive linear layers (no nonlinearity between them):
    # y2 = silu(attn @ Wa + ba); y4 = silu(y2 @ Wb + bb); y5 = silu(y4 @ Wd1 + bd1)
    Wa = Wvo @ f["Wt0"]
    ba = f["bo"] @ f["Wt0"] + f["bt0"]
    Wb = f["Wt1"] @ f["Wd0"]
    bb = f["bt1"] @ f["Wd0"] + f["bd0"]

    fl = lambda x: np.ascontiguousarray(x, np.float32)
    return {
        "SA": fl(SA), "Sw": fl(Sw), "Sb": fl(Sb), "mu": fl(mu), "msq": fl(msq),
        "orbT_s": fl(orb.T * np.sqrt(2.0 / D)),                          # (D, NB)
        "Wa": fl(Wa), "ba": fl(ba), "Wb": fl(Wb), "bb": fl(bb),
        "Wd1": fl(f["Wd1"]), "bd1": fl(f["bd1"]),
        "Wd2": fl(f["Wd2"]), "bd2": fl(f["bd2"]),
    }


def core_rows(c):
    return [B * 128 + r * NCORES + c for B in range(NBLK) for r in range(RPB)]


def _core_inputs(pc, c):
    rows = core_rows(c)
    ones80 = np.ones(NROWS, np.float32)
    return {
        "sa_in": pc["SA"],

        "orbT_in": pc["orbT_s"],
        "orbTc_in": np.ascontiguousarray(pc["orbT_s"][:, rows]),
        "lhs_mu": np.ascontiguousarray(np.stack([ones80, pc["mu"][rows]])),
        "lhs_msq": np.ascontiguousarray(np.stack([ones80, pc["msq"][rows]])),
        "rhs_mu": np.ascontiguousarray(np.stack([pc["mu"], np.ones(NB, np.float32)])),
        "rhs_msq": np.ascontiguousarray(np.stack([pc["msq"], np.ones(NB, np.float32)])),
        "r3_all": np.ascontiguousarray(np.stack(
            [np.stack([pc["SA"][i], -pc["Sw"], pc["Sb"]]) for i in rows])),
        "ident_in": np.eye(128, dtype=np.float32),
        "ones_in": np.ones((1, NB), np.float32),
        "wa": pc["Wa"], "wb": pc["Wb"], "wd1": pc["Wd1"], "wd2": pc["Wd2"],
        "ba_in": pc["ba"], "bb_in": pc["bb"],
        "bd1_in": pc["bd1"], "bd2_in": pc["bd2"],
    }


def _build_nc(n_chunks):
    import concourse.bass as bass
    import concourse.bacc as bacc
    import concourse.tile as tile
    from concourse import mybir
    dt = mybir.dt
    f32 = dt.float32
    f32r = dt.float32r
    AF = mybir.ActivationFunctionType
    AX = mybir.AxisListType

    nc = bacc.Bacc(None, target_bir_lowering=False)

    ein = lambda name, shape, d=f32: nc.dram_tensor(name, shape, d,
                                                     kind="ExternalInput")
    sa_in = ein("sa_in", [NB, 256], f32r)
    orbT_in = ein("orbT_in", [D, NB], f32r)
    orbTc_in = ein("orbTc_in", [D, NROWS], f32r)
    lhs_mu = ein("lhs_mu", [2, NROWS], f32r)
    lhs_msq = ein("lhs_msq", [2, NROWS], f32r)
    rhs_mu = ein("rhs_mu", [2, NB], f32r)
    rhs_msq = ein("rhs_msq", [2, NB], f32r)
    r3_all = ein("r3_all", [NROWS, 3, 256], f32r)
    ident_in = ein("ident_in", [128, 128], f32r)
    ones_in = ein("ones_in", [1, NB], f32r)
    wa = ein("wa", [256, 256], f32r)
    wb = ein("wb", [256, 256], f32r)
    wd1 = ein("wd1", [256, 256], f32r)
    wd2 = ein("wd2", [256, 2], f32r)
    ba_in = ein("ba_in", [256])
    bb_in = ein("bb_in", [256])
    bd1_in = ein("bd1_in", [256])
    bd2_in = ein("bd2_in", [2])

    out_ext = nc.dram_tensor("out", [NCHUNKS, 2, 512], f32, kind="ExternalOutput")
    stats_dram = nc.dram_tensor("stats_scratch", [2, NROWS, NB], f32r)

    with tile.TileContext(nc) as tc, \
            nc.allow_low_precision(reason="fp32r matmul pipeline by design"):
        with (
            tc.tile_pool(name="const", bufs=1) as const,
            tc.tile_pool(name="prow", bufs=2) as prow,
            tc.tile_pool(name="score", bufs=int(os.environ.get("DD_SCORE", "5"))) as score,
            tc.tile_pool(name="small", bufs=5) as small,
            tc.tile_pool(name="attnT", bufs=2 * GROUP + 3) as attnT_pool,
            tc.tile_pool(name="chainx", bufs=int(os.environ.get("DD_CHX", "2"))) as chainx,
        ):
            # ---- constants into SBUF ----
            sa = const.tile([128, NBLK, 256], f32r)
            nc.sync.dma_start(out=sa, in_=sa_in.rearrange("(jt p) c -> p jt c", p=128))
            orbT = const.tile([128, 2, NB], f32r)
            nc.sync.dma_start(out=orbT, in_=orbT_in.rearrange("(k p) n -> p k n", p=128))
            orbTc = const.tile([128, 2, NROWS], f32r)
            nc.sync.dma_start(out=orbTc, in_=orbTc_in.rearrange("(k p) m -> p k m", p=128))
            lmu = const.tile([2, NROWS], f32r)
            nc.sync.dma_start(out=lmu, in_=lhs_mu[:])
            lmsq = const.tile([2, NROWS], f32r)
            nc.sync.dma_start(out=lmsq, in_=lhs_msq[:])
            rmu = const.tile([2, NB], f32r)
            nc.sync.dma_start(out=rmu, in_=rhs_mu[:])
            rmsq = const.tile([2, NB], f32r)
            nc.sync.dma_start(out=rmsq, in_=rhs_msq[:])

            w_a = const.tile([128, 2, 256], f32r)
            nc.sync.dma_start(out=w_a, in_=wa.rearrange("(k p) n -> p k n", p=128))
            w_b = const.tile([128, 2, 256], f32r)
            nc.sync.dma_start(out=w_b, in_=wb.rearrange("(k p) n -> p k n", p=128))
            w_d1 = const.tile([128, 2, 256], f32r)
            nc.sync.dma_start(out=w_d1, in_=wd1.rearrange("(k p) n -> p k n", p=128))
            w_d2 = const.tile([128, 2, 2], f32r)
            nc.sync.dma_start(out=w_d2, in_=wd2.rearrange("(k p) n -> p k n", p=128))

            b_a = const.tile([128, 2], f32)
            nc.sync.dma_start(out=b_a, in_=ba_in.rearrange("(m p) -> p m", p=128))
            b_b = const.tile([128, 2], f32)
            nc.sync.dma_start(out=b_b, in_=bb_in.rearrange("(m p) -> p m", p=128))
            b_d1 = const.tile([128, 2], f32)
            nc.sync.dma_start(out=b_d1, in_=bd1_in.rearrange("(m p) -> p m", p=128))
            b_d2 = const.tile([2, 1], f32)
            nc.sync.dma_start(out=b_d2, in_=bd2_in.rearrange("(p one) -> p one", one=1))

            ident = const.tile([128, 128], f32r)
            nc.sync.dma_start(out=ident, in_=ident_in[:])
            l3_bufs = [const.tile([3, NB], f32r, tag=f"l3_{i}", name=f"l3_{i}")
                       for i in range(2)]
            for lb in l3_bufs:
                nc.sync.dma_start(out=lb[0:1, :], in_=ones_in[:])
            eps_t = const.tile([NROWS, 1], f32)
            nc.gpsimd.memset(eps_t, EPS)

            # persistent per-row stats
            mu_p_sb = const.tile([NROWS, NB], f32r)
            invr_sb = const.tile([NROWS, NB], f32r)
            rstd_sb = const.tile([NROWS, NB], f32r)
            rstd_T = const.tile([128, NBLK, NROWS], f32)

            # ---- prologue: per-pair LN stats for this core's 80 rows ----
            with (
                tc.tile_pool(name="pro_ps", bufs=2, space="PSUM") as pro_ps,
                tc.tile_pool(name="pro_sb", bufs=2) as pro_sb,
            ):
                for nch in range(2):
                    seg = slice(nch * 320, (nch + 1) * 320)
                    psA = pro_ps.tile([NROWS, 320], f32, tag="psA")
                    nc.tensor.matmul(psA, lmu, rmu[:, seg], start=True, stop=True)
                    nc.vector.tensor_copy(out=mu_p_sb[:, seg], in_=psA)
                    psB = pro_ps.tile([NROWS, 320], f32, tag="psB")
                    nc.tensor.matmul(psB, lmsq, rmsq[:, seg], start=True, stop=False)
                    nc.tensor.matmul(psB, orbTc[:, 0, :], orbT[:, 0, seg],
                                     start=False, stop=False)
                    nc.tensor.matmul(psB, orbTc[:, 1, :], orbT[:, 1, seg],
                                     start=False, stop=True)
                    mu2 = pro_sb.tile([NROWS, 320], f32, tag="mu2")
                    nc.vector.tensor_mul(mu2, mu_p_sb[:, seg], mu_p_sb[:, seg])
                    nc.vector.tensor_sub(invr_sb[:, seg], psB, mu2)
                # invr = sqrt(var + eps); rstd = 1/invr
                nc.scalar.activation(out=invr_sb, in_=invr_sb, func=AF.Sqrt,
                                     bias=eps_t[:, 0:1])
                nc.vector.reciprocal(out=rstd_sb, in_=invr_sb)
                nc.sync.dma_start(out=stats_dram[0], in_=mu_p_sb)
                nc.sync.dma_start(out=stats_dram[1], in_=invr_sb)
                for jt in range(NBLK):
                    pT = pro_ps.tile([128, NROWS], f32r, tag="pT")
                    nc.tensor.transpose(
                        pT, rstd_sb[:, jt * 128:(jt + 1) * 128],
                        ident[0:NROWS, 0:NROWS])
                    nc.vector.tensor_copy(out=rstd_T[:, jt, :], in_=pT)

            # ---- main loop: superchunks separate Exp (phase A) from Silu
            # (phase B) on the scalar engine. Each activation-function switch
            # costs a ~1.3us InstLoadActFuncSet table load, so ACT program
            # order is pinned with order-only deps: [A exps][B silus] per
            # superchunk -> 2 table loads per superchunk instead of per tile.
            # Other engines still overlap phase B(s) with phase A(s+1).
            from concourse.tile_rust import add_dep_helper
            import contextlib
            _mstack = contextlib.ExitStack()
            px3_pool = _mstack.enter_context(
                tc.tile_pool(name="px3", bufs=int(os.environ.get("DD_PX3", "2")), space="PSUM"))
            ptr_pool = _mstack.enter_context(
                tc.tile_pool(name="ptr", bufs=int(os.environ.get("DD_PTR", "1")), space="PSUM"))
            pchain = _mstack.enter_context(
                tc.tile_pool(name="pchain", bufs=int(os.environ.get("DD_PCH", "2")), space="PSUM"))
            repeat = int(os.environ.get("DD_REPEAT", "1"))
            skip = os.environ.get("DD_SKIP", "")

            act_prev = [None]
            nopin = bool(int(os.environ.get("DD_NOPIN", "0")))

            def act_chain(bi):
                if act_prev[0] is not None and not nopin:
                    add_dep_helper(bi.ins, act_prev[0].ins, sync=True,
                                   reason="pin ACT order for act-table reuse")
                act_prev[0] = bi
                return bi

            prev_row = [None, None, None]   # r_loc, l3row, r3

            def score_pair(t0, ptrt):
                # two consecutive tiles share one ee/attn buffer so the
                # softmax reduce / reciprocal / normalize run as single ops
                # (and one PSUM bank holds both tiles' scores)
                ee = score.tile([128, 2, 8, 32], f32, tag="ee", name="ee")
                for ti in range(2):
                    t = t0 + ti
                    B, r, jt = TILES[t]
                    r_loc = B * RPB + r
                    if prev_row[0] != r_loc:
                        prev_row[0] = r_loc
                        # lhsT rows: [ones; mu_p(row); invrstd(row)];
                        # rhs rows: [SA_i; -Sw; Sb]
                        l3row = l3_bufs[r_loc % 2]
                        nc.sync.dma_start(out=l3row[1:3, :],
                                          in_=stats_dram[:, r_loc, :])
                        r3 = prow.tile([3, 256], f32r, tag="r3", name="r3")
                        nc.sync.dma_start(out=r3, in_=r3_all[r_loc])
                        prev_row[1], prev_row[2] = l3row, r3
                    l3row, r3 = prev_row[1], prev_row[2]
                    jseg = slice(jt * 128, (jt + 1) * 128)
                    # scores-pre-rstd accumulated fully on PE:
                    #   px3 = SA_i - mu*Sw + invr*Sb  (rank-3)  +  I @ SA_j
                    px3 = px3_pool.tile([128, 256], f32, tag="px3", name="px3")
                    nc.tensor.matmul(px3, l3row[:, jseg], r3,
                                     start=True, stop=False)
                    nc.tensor.matmul(px3, ident, sa[:, jt, :],
                                     start=False, stop=True)
                    # E = exp(rstd * scores)
                    act_chain(nc.scalar.activation(
                        out=ee[:, ti, :, :].rearrange("p h t -> p (h t)"),
                        in_=px3,
                        func=AF.Exp,
                        scale=rstd_T[:, jt, r_loc:r_loc + 1]))
                den = small.tile([128, 2, 8], f32, tag="den", name="den")
                nc.vector.reduce_sum(out=den, in_=ee, axis=AX.X)
                rden = small.tile([128, 2, 8], f32, tag="rden", name="rden")
                nc.vector.reciprocal(out=rden, in_=den)
                attn = score.tile([128, 2, 8, 32], f32r, tag="attn", name="attn")
                nc.gpsimd.tensor_mul(attn, ee,
                                     rden.to_broadcast([128, 2, 8, 32]))
                if skip == "chain2":
                    return
                for ti in range(2):
                    s = (t0 + ti) % CHUNK
                    a2 = attn[:, ti, :, :].rearrange("p h t -> p (h t)")
                    sseg = slice(s * 128, (s + 1) * 128)
                    nc.tensor.transpose(ptrt[:, 0, sseg], a2[:, 0:128], ident)
                    nc.tensor.transpose(ptrt[:, 1, sseg], a2[:, 128:256], ident)

            unpair = bool(int(os.environ.get("DD_UNPAIR", "0")))

            def chain_pair(aT_pair, q_pair):
                if unpair:
                    # single-chunk chain: [128,512] psums (1 bank each)
                    def layer1(x_kt, w, b_tile, out_tile, qi):
                        for mt in range(2):
                            ps = pchain.tile([128, 512], f32, tag="pch",
                                             name="pch")
                            for kt in range(2):
                                nc.tensor.matmul(
                                    ps, w[:, kt, mt * 128:(mt + 1) * 128],
                                    x_kt(kt), start=(kt == 0), stop=(kt == 1))
                            act_chain(nc.scalar.activation(
                                out=out_tile[:, mt, qi, :], in_=ps,
                                func=AF.Silu, bias=b_tile[:, mt:mt + 1]))
                    x2 = chainx.tile([128, 2, 2, 512], f32r, tag="x2", name="x2")
                    x4 = chainx.tile([128, 2, 2, 512], f32r, tag="x4", name="x4")
                    x5 = chainx.tile([128, 2, 2, 512], f32r, tag="x5", name="x5")
                    o6 = small.tile([2, 2, 512], f32, tag="o6", name="o6")
                    for qi in range(2):
                        layer1(lambda kt: aT_pair[qi][:, kt, :], w_a, b_a, x2, qi)
                        layer1(lambda kt: x2[:, kt, qi, :], w_b, b_b, x4, qi)
                        layer1(lambda kt: x4[:, kt, qi, :], w_d1, b_d1, x5, qi)
                        ps6 = pchain.tile([2, 512], f32, tag="pch", name="ps6")
                        for kt in range(2):
                            nc.tensor.matmul(ps6, w_d2[:, kt, :],
                                             x5[:, kt, qi, :],
                                             start=(kt == 0), stop=(kt == 1))
                        nc.vector.tensor_copy(out=o6[:, qi, :], in_=ps6)
                        nc.sync.dma_start(out=out_ext[q_pair[qi]], in_=o6[:, qi, :])
                    return
                # two chunks share each silu: psum [128, 2, 512] spans two
                # banks, one [128, 1024] activation per (layer, mt) halves
                # the scalar engine's fixed per-op cost.
                def layer(x_of, w, b_tile, out_tile):
                    for mt in range(2):
                        ps = pchain.tile([128, 2, 512], f32, tag="pch",
                                         name="pch")
                        for qi in range(2):
                            for kt in range(2):
                                nc.tensor.matmul(
                                    ps[:, qi, :],
                                    w[:, kt, mt * 128:(mt + 1) * 128],
                                    x_of(qi, kt),
                                    start=(kt == 0), stop=(kt == 1))
                        act_chain(nc.scalar.activation(
                            out=out_tile[:, mt, :, :].rearrange(
                                "p q n -> p (q n)"),
                            in_=ps.rearrange("p q n -> p (q n)"), func=AF.Silu,
                            bias=b_tile[:, mt:mt + 1]))

                x2 = chainx.tile([128, 2, 2, 512], f32r, tag="x2", name="x2")
                layer(lambda qi, kt: aT_pair[qi][:, kt, :], w_a, b_a, x2)
                x4 = chainx.tile([128, 2, 2, 512], f32r, tag="x4", name="x4")
                layer(lambda qi, kt: x2[:, kt, qi, :], w_b, b_b, x4)
                x5 = chainx.tile([128, 2, 2, 512], f32r, tag="x5", name="x5")
                layer(lambda qi, kt: x4[:, kt, qi, :], w_d1, b_d1, x5)
                ps6 = pchain.tile([2, 2, 512], f32, tag="pch", name="ps6")
                for qi in range(2):
                    for kt in range(2):
                        nc.tensor.matmul(ps6[:, qi, :], w_d2[:, kt, :],
                                         x5[:, kt, qi, :],
                                         start=(kt == 0), stop=(kt == 1))
                # bias bd2 is added host-side during assembly
                o6 = small.tile([2, 2, 512], f32, tag="o6", name="o6")
                nc.vector.tensor_copy(out=o6, in_=ps6)
                for qi in range(2):
                    nc.sync.dma_start(out=out_ext[q_pair[qi]], in_=o6[:, qi, :])

            n_super = (n_chunks + GROUP - 1) // GROUP

            def phase_A(sc):
                qs = list(range(sc * GROUP, min((sc + 1) * GROUP, n_chunks)))
                aTs = []
                for q in qs:
                    ptrt = ptr_pool.tile([128, 2, 512], f32r, tag="ptrt",
                                         name="ptrt")
                    for s in range(0, CHUNK, 2):
                        score_pair(q * CHUNK + s, ptrt)
                    if skip in ("chain", "chain2"):
                        continue
                    aT = attnT_pool.tile([128, 2, 512], f32r, tag="aT",
                                         name="aT")
                    nc.vector.tensor_copy(out=aT, in_=ptrt)
                    aTs.append(aT)
                return qs, aTs

            for rep in range(repeat):
                prev_row[0] = None
                # chains run one superchunk behind scores, so the pinned ACT
                # order [exps(s)][exps(s+1)][silus(s)]... never stalls the
                # score pipeline on chain completion.
                def run_chains(p):
                    qs, aTs = p
                    for i in range(0, len(qs) - 1, 2):
                        chain_pair(aTs[i:i + 2], qs[i:i + 2])

                pending = None
                for sc in range(n_super):
                    qa = phase_A(sc)
                    if pending and skip not in ("chain", "chain2"):
                        run_chains(pending)
                    pending = qa
                if pending and skip not in ("chain", "chain2"):
                    run_chains(pending)
            _mstack.close()
    nc.compile()
    return nc


def _get_nc(n_chunks):
    key = ("nc", n_chunks)
    if key not in _CACHE:
        _CACHE[key] = _build_nc(n_chunks)
    return _CACHE[key]


def kernel(**inputs):
    from concourse.bass_utils import run_bass_kernel_spmd

    n_chunks = int(os.environ.get("DD_CHUNKS", NCHUNKS))
    pc = _precompute(inputs)
    in_maps = [_core_inputs(pc, c) for c in range(NCORES)]
    nc = _get_nc(n_chunks)
    res = run_bass_kernel_spmd(nc, in_maps, core_ids=list(range(NCORES)),
                               trace=bool(int(os.environ.get("DD_TRACE", "0"))))
    _CACHE["last_result"] = res

    R = np.zeros((NB, NB, 2), np.float32)
    for c in range(NCORES):
        o = res.results[c]["out"] + pc["bd2"][None, :, None]   # (NCHUNKS, 2, 512)
        ot = o.reshape(NCHUNKS, 2, CHUNK, 128).transpose(0, 2, 1, 3).reshape(-1, 2, 128)
        for t in range(n_chunks * CHUNK):
            B, r, jt = TILES[t]
            i = B * 128 + r * NCORES + c
            R[i, jt * 128:(jt + 1) * 128, 0] = ot[t, 0]
            R[i, jt * 128:(jt + 1) * 128, 1] = ot[t, 1]
    for bi in range(NBLK):
        for bj in range(bi):
            R[bi * 128:(bi + 1) * 128, bj * 128:(bj + 1) * 128] = \
                R[bj * 128:(bj + 1) * 128, bi * 128:(bi + 1) * 128].transpose(1, 0, 2)

    rho = (R[:, :, 0] + 1j * R[:, :, 1]).astype(np.complex64)
    n_spin = int(np.asarray(inputs["n_spin"]))
    return np.broadcast_to(rho[None], (n_spin, NB, NB)).copy()



# revision 2
# speedup vs baseline: 1.6797x; 1.6797x over previous
"""Trainium2 Bass kernel for nn_DensityDecoder (gnn_message_passing).

Math: the reference computes, for every ordered pair (i, j) of NB=640 orbitals,
    pair = orb_i + orb_j                       (orb: per-orbital projected embedding)
    qn   = LayerNorm(pair) ; q = qn @ Wq + bq
    attn = softmax(q . k / sqrt(Dh)) over a tiny T=32 latent KV
    out  = MLP(attn @ V @ Wo)  ->  2 values -> rho[i, j] = out0 + 1j*out1

Because pair = orb_i + orb_j, the LN statistics decompose exactly:
    mu_ij  = mu_i + mu_j
    var_ij = msq_i + msq_j + 2*G_ij - mu_ij^2        (G = orb @ orb.T / D)
and the whole pre-softmax pipeline collapses to per-orbital precomputes
(SA = ((orb*g) @ Wq) projected into (head, token) score space, plus the
constant vectors Sw, Sb), so the 410MB pair tensor is never materialized:
    scores_ij = rstd_ij * (SA_i + SA_j - mu_ij*Sw) + Sb      (pre-scaled by 1/sqrt(Dh))
The per-pair device work is the softmax + a 5-layer MLP chain, where
attn @ V @ Wo is folded into one 256x256 matmul (Wvo = blockdiag(V) @ Wo).

Dedup: orb_i depends only on the (Z, l, m) triple, so rows with equal triples
are identical.  The device computes rho over the U distinct classes only
(U=396 for the reference inputs vs NB=640), and the host scatters
rho[i, j] = rho_class[cls[i], cls[j]].  rho_class is symmetric, so only
class pairs a <= b are computed; j is tiled into 128-wide virtual blocks
[0,128), [128,256), ..., [U-128, U) (last block right-aligned so padding
never exceeds one block), and class-rows are grouped by the j-blocks they
need: rows in [128g, 128(g+1)) take blocks g..NBLKJ-1, rows in the last
block take only the final one.

Sharding: class-rows are striped across the 8 cores (slot % 8 == core) within
each group, giving every core an IDENTICAL instruction stream (same NEFF,
SPMD) over different data.
"""

import os
import numpy as np

EPS = 1e-5
H = 8
D = 256
T = 32
NB = 640
NCORES = 8
CHUNK = 4                 # tiles per MLP-chain chunk (512 pair columns)
GROUP = int(os.environ.get("DD_GROUP", "8"))  # chunks per superchunk

_CACHE = {}


def _silu(x):
    return x / (1.0 + np.exp(-x))


def _ln(x, g, b):
    mu = x.mean(-1, keepdims=True)
    var = x.var(-1, keepdims=True)
    return (x - mu) / np.sqrt(var + EPS) * g + b


def _structure(U):
    """Tile structure over U distinct classes.

    Returns (NROWSC, NBLKJ, vmap, core_rows, TILES, NCHUNKS, ntiles_real):
      vmap[v]       class index of virtual j column v (NBLKJ*128 columns)
      core_rows[c]  class index of each row slot on core c (NROWSC slots)
      TILES         [(row_slot, jt)] identical on every core
    """
    Upad = max(U, 128)
    NBLKJ = -(-Upad // 128)
    assert NBLKJ <= 8, "stats psum segment would exceed a PSUM bank"
    starts = [128 * k for k in range(NBLKJ - 1)] + [Upad - 128]
    bounds = starts + [Upad]
    vmap = np.concatenate(
        [np.clip(np.arange(s, s + 128), 0, U - 1) for s in starts])

    core_rows = [[] for _ in range(NCORES)]
    tiles = []
    for g in range(NBLKJ):
        rows = [min(r, U - 1) for r in range(bounds[g], bounds[g + 1])]
        while len(rows) % NCORES:
            rows.append(rows[0])
        nloc = len(rows) // NCORES
        base = len(core_rows[0])
        for c in range(NCORES):
            core_rows[c].extend(rows[c::NCORES])
        for sl in range(nloc):
            for jt in range(g, NBLKJ):
                tiles.append((base + sl, jt))
    ntiles_real = len(tiles)
    while len(tiles) % (2 * CHUNK):
        tiles.append(tiles[-1])
    NROWSC = len(core_rows[0])
    NCHUNKS = len(tiles) // CHUNK
    return NROWSC, NBLKJ, vmap, core_rows, tiles, NCHUNKS, ntiles_real


def _precompute(inputs):
    """Class-level precompute (all O(U*D) or smaller), numpy float64 -> float32."""
    f = {}
    for k, v in inputs.items():
        v = np.asarray(v)
        f[k] = v.astype(np.float64) if v.dtype in (np.float32, np.float64) else v
    Z = np.asarray(inputs["Z"]).astype(np.int64)
    l = np.asarray(inputs["l"]).astype(np.int64)
    m = np.asarray(inputs["m"]).astype(np.int64)
    m_idx = np.clip(m + 3, 0, 4)
    key = np.stack([Z, l, m_idx], 1)
    uniq, inverse = np.unique(key, axis=0, return_inverse=True)
    U = len(uniq)

    emb = np.concatenate(
        [f["elem_tab"][uniq[:, 0]], f["l_tab"][uniq[:, 1]], f["m_tab"][uniq[:, 2]]], -1)
    orb = _silu(emb @ f["Wp0"] + f["bp0"]) @ f["Wp1"] + f["bp1"]          # (U, D)

    kv = _ln(f["latent"], f["ln_gkv"], f["ln_bkv"])
    k = (kv @ f["Wk"] + f["bk"]).reshape(T, H, D // H)
    v = (kv @ f["Wv"] + f["bv"]).reshape(T, H, D // H)

    g, b = f["ln_gq"], f["ln_bq"]
    mu = orb.mean(-1)
    msq = (orb * orb).mean(-1)

    A = (orb * g) @ f["Wq"]
    wbar = g @ f["Wq"]
    bq_eff = b @ f["Wq"] + f["bqa"]

    kT = k.transpose(1, 2, 0)                                            # (H, Dh, T)
    scale = 1.0 / np.sqrt(np.float64(D // H))

    def to_scores(x):
        xh = x.reshape(x.shape[:-1] + (H, D // H))
        return (np.einsum('...hd,hdt->...ht', xh, kT).reshape(x.shape[:-1] + (H * T,))
                * scale)

    SA = to_scores(A)                                                    # (U, 256)
    Sw = to_scores(wbar)                                                 # (256,)
    Sb = to_scores(bq_eff)                                               # (256,)
    Wvo = np.einsum('thd,hde->hte', v, f["Wo"].reshape(H, D // H, D)).reshape(H * T, D)
    # fuse consecutive linear layers (no nonlinearity between them):
    # y2 = silu(attn @ Wa + ba); y4 = silu(y2 @ Wb + bb); y5 = silu(y4 @ Wd1 + bd1)
    Wa = Wvo @ f["Wt0"]
    ba = f["bo"] @ f["Wt0"] + f["bt0"]
    Wb = f["Wt1"] @ f["Wd0"]
    bb = f["bt1"] @ f["Wd0"] + f["bd0"]

    fl = lambda x: np.ascontiguousarray(x, np.float32)
    return {
        "U": U, "inverse": inverse,
        "SA": fl(SA), "Sw": fl(Sw), "Sb": fl(Sb), "mu": fl(mu), "msq": fl(msq),
        "orbT_s": fl(orb.T * np.sqrt(2.0 / D)),                          # (D, U)
        "Wa": fl(Wa), "ba": fl(ba), "Wb": fl(Wb), "bb": fl(bb),
        "Wd1": fl(f["Wd1"]), "bd1": fl(f["bd1"]),
        "Wd2": fl(f["Wd2"]), "bd2": fl(f["bd2"]),
    }


def _core_inputs(pc, st, c):
    NROWSC, NBLKJ, vmap, core_rows, tiles, NCHUNKS, _ = st
    rows = core_rows[c]
    ones_r = np.ones(NROWSC, np.float32)
    ones_v = np.ones(len(vmap), np.float32)
    return {
        "sa_in": np.ascontiguousarray(pc["SA"][vmap]),
        "orbT_in": np.ascontiguousarray(pc["orbT_s"][:, vmap]),
        "orbTc_in": np.ascontiguousarray(pc["orbT_s"][:, rows]),
        "lhs_mu": np.ascontiguousarray(np.stack([ones_r, pc["mu"][rows]])),
        "lhs_msq": np.ascontiguousarray(np.stack([ones_r, pc["msq"][rows]])),
        "rhs_mu": np.ascontiguousarray(np.stack([pc["mu"][vmap], ones_v])),
        "rhs_msq": np.ascontiguousarray(np.stack([pc["msq"][vmap], ones_v])),
        "r3_all": np.ascontiguousarray(np.stack(
            [np.stack([pc["SA"][i], -pc["Sw"], pc["Sb"]]) for i in rows])),
        "ident_in": np.eye(128, dtype=np.float32),
        "ones_in": np.ones((1, NBLKJ * 128), np.float32),
        "wa": pc["Wa"], "wb": pc["Wb"], "wd1": pc["Wd1"], "wd2": pc["Wd2"],
        "ba_in": pc["ba"], "bb_in": pc["bb"],
        "bd1_in": pc["bd1"], "bd2_in": pc["bd2"],
    }


def _build_nc(st):
    NROWSC, NBLKJ, _vmap, _core_rows, TILES, NCHUNKS, _ = st
    NBV = NBLKJ * 128
    import concourse.bass as bass
    import concourse.bacc as bacc
    import concourse.tile as tile
    from concourse import mybir
    dt = mybir.dt
    f32 = dt.float32
    f32r = dt.float32r
    AF = mybir.ActivationFunctionType
    AX = mybir.AxisListType

    nc = bacc.Bacc(None, target_bir_lowering=False)

    ein = lambda name, shape, d=f32: nc.dram_tensor(name, shape, d,
                                                     kind="ExternalInput")
    sa_in = ein("sa_in", [NBV, 256], f32r)
    orbT_in = ein("orbT_in", [D, NBV], f32r)
    orbTc_in = ein("orbTc_in", [D, NROWSC], f32r)
    lhs_mu = ein("lhs_mu", [2, NROWSC], f32r)
    lhs_msq = ein("lhs_msq", [2, NROWSC], f32r)
    rhs_mu = ein("rhs_mu", [2, NBV], f32r)
    rhs_msq = ein("rhs_msq", [2, NBV], f32r)
    r3_all = ein("r3_all", [NROWSC, 3, 256], f32r)
    ident_in = ein("ident_in", [128, 128], f32r)
    ones_in = ein("ones_in", [1, NBV], f32r)
    wa = ein("wa", [256, 256], f32r)
    wb = ein("wb", [256, 256], f32r)
    wd1 = ein("wd1", [256, 256], f32r)
    wd2 = ein("wd2", [256, 2], f32r)
    ba_in = ein("ba_in", [256])
    bb_in = ein("bb_in", [256])
    bd1_in = ein("bd1_in", [256])
    bd2_in = ein("bd2_in", [2])

    out_ext = nc.dram_tensor("out", [NCHUNKS, 2, 512], f32, kind="ExternalOutput")
    stats_dram = nc.dram_tensor("stats_scratch", [2, NROWSC, NBV], f32r)

    with tile.TileContext(nc) as tc, \
            nc.allow_low_precision(reason="fp32r matmul pipeline by design"):
        with (
            tc.tile_pool(name="const", bufs=1) as const,
            tc.tile_pool(name="prow", bufs=2) as prow,
            tc.tile_pool(name="score", bufs=int(os.environ.get("DD_SCORE", "5"))) as score,
            tc.tile_pool(name="small", bufs=5) as small,
            tc.tile_pool(name="attnT", bufs=2 * GROUP + 3) as attnT_pool,
            tc.tile_pool(name="chainx", bufs=int(os.environ.get("DD_CHX", "2"))) as chainx,
        ):
            # ---- constants into SBUF ----
            sa = const.tile([128, NBLKJ, 256], f32r)
            nc.sync.dma_start(out=sa, in_=sa_in.rearrange("(jt p) c -> p jt c", p=128))
            orbT = const.tile([128, 2, NBV], f32r)
            nc.sync.dma_start(out=orbT, in_=orbT_in.rearrange("(k p) n -> p k n", p=128))
            orbTc = const.tile([128, 2, NROWSC], f32r)
            nc.sync.dma_start(out=orbTc, in_=orbTc_in.rearrange("(k p) m -> p k m", p=128))
            lmu = const.tile([2, NROWSC], f32r)
            nc.sync.dma_start(out=lmu, in_=lhs_mu[:])
            lmsq = const.tile([2, NROWSC], f32r)
            nc.sync.dma_start(out=lmsq, in_=lhs_msq[:])
            rmu = const.tile([2, NBV], f32r)
            nc.sync.dma_start(out=rmu, in_=rhs_mu[:])
            rmsq = const.tile([2, NBV], f32r)
            nc.sync.dma_start(out=rmsq, in_=rhs_msq[:])

            w_a = const.tile([128, 2, 256], f32r)
            nc.sync.dma_start(out=w_a, in_=wa.rearrange("(k p) n -> p k n", p=128))
            w_b = const.tile([128, 2, 256], f32r)
            nc.sync.dma_start(out=w_b, in_=wb.rearrange("(k p) n -> p k n", p=128))
            w_d1 = const.tile([128, 2, 256], f32r)
            nc.sync.dma_start(out=w_d1, in_=wd1.rearrange("(k p) n -> p k n", p=128))
            w_d2 = const.tile([128, 2, 2], f32r)
            nc.sync.dma_start(out=w_d2, in_=wd2.rearrange("(k p) n -> p k n", p=128))

            b_a = const.tile([128, 2], f32)
            nc.sync.dma_start(out=b_a, in_=ba_in.rearrange("(m p) -> p m", p=128))
            b_b = const.tile([128, 2], f32)
            nc.sync.dma_start(out=b_b, in_=bb_in.rearrange("(m p) -> p m", p=128))
            b_d1 = const.tile([128, 2], f32)
            nc.sync.dma_start(out=b_d1, in_=bd1_in.rearrange("(m p) -> p m", p=128))
            b_d2 = const.tile([2, 1], f32)
            nc.sync.dma_start(out=b_d2, in_=bd2_in.rearrange("(p one) -> p one", one=1))

            ident = const.tile([128, 128], f32r)
            nc.sync.dma_start(out=ident, in_=ident_in[:])
            l3_bufs = [const.tile([3, NBV], f32r, tag=f"l3_{i}", name=f"l3_{i}")
                       for i in range(2)]
            for lb in l3_bufs:
                nc.sync.dma_start(out=lb[0:1, :], in_=ones_in[:])
            eps_t = const.tile([NROWSC, 1], f32)
            nc.gpsimd.memset(eps_t, EPS)

            # persistent per-row stats
            mu_p_sb = const.tile([NROWSC, NBV], f32r)
            invr_sb = const.tile([NROWSC, NBV], f32r)
            rstd_sb = const.tile([NROWSC, NBV], f32r)
            rstd_T = const.tile([128, NBLKJ, NROWSC], f32)

            # ---- prologue: per-pair LN stats for this core's rows ----
            with (
                tc.tile_pool(name="pro_ps", bufs=2, space="PSUM") as pro_ps,
                tc.tile_pool(name="pro_sb", bufs=2) as pro_sb,
            ):
                seg_w = NBV // 2
                for nch in range(2):
                    seg = slice(nch * seg_w, (nch + 1) * seg_w)
                    psA = pro_ps.tile([NROWSC, seg_w], f32, tag="psA")
                    nc.tensor.matmul(psA, lmu, rmu[:, seg], start=True, stop=True)
                    nc.vector.tensor_copy(out=mu_p_sb[:, seg], in_=psA)
                    psB = pro_ps.tile([NROWSC, seg_w], f32, tag="psB")
                    nc.tensor.matmul(psB, lmsq, rmsq[:, seg], start=True, stop=False)
                    nc.tensor.matmul(psB, orbTc[:, 0, :], orbT[:, 0, seg],
                                     start=False, stop=False)
                    nc.tensor.matmul(psB, orbTc[:, 1, :], orbT[:, 1, seg],
                                     start=False, stop=True)
                    mu2 = pro_sb.tile([NROWSC, seg_w], f32, tag="mu2")
                    nc.vector.tensor_mul(mu2, mu_p_sb[:, seg], mu_p_sb[:, seg])
                    nc.vector.tensor_sub(invr_sb[:, seg], psB, mu2)
                # invr = sqrt(var + eps); rstd = 1/invr
                nc.scalar.activation(out=invr_sb, in_=invr_sb, func=AF.Sqrt,
                                     bias=eps_t[:, 0:1])
                nc.vector.reciprocal(out=rstd_sb, in_=invr_sb)
                nc.sync.dma_start(out=stats_dram[0], in_=mu_p_sb)
                nc.sync.dma_start(out=stats_dram[1], in_=invr_sb)
                for jt in range(NBLKJ):
                    pT = pro_ps.tile([128, NROWSC], f32r, tag="pT")
                    nc.tensor.transpose(
                        pT, rstd_sb[:, jt * 128:(jt + 1) * 128],
                        ident[0:NROWSC, 0:NROWSC])
                    nc.vector.tensor_copy(out=rstd_T[:, jt, :], in_=pT)

            # ---- main loop: superchunks separate Exp (phase A) from Silu
            # (phase B) on the scalar engine. Each activation-function switch
            # costs a ~1.3us InstLoadActFuncSet table load, so ACT program
            # order is pinned with order-only deps: [A exps][B silus] per
            # superchunk -> 2 table loads per superchunk instead of per tile.
            # Other engines still overlap phase B(s) with phase A(s+1).
            from concourse.tile_rust import add_dep_helper
            import contextlib
            _mstack = contextlib.ExitStack()
            px3_pool = _mstack.enter_context(
                tc.tile_pool(name="px3", bufs=int(os.environ.get("DD_PX3", "2")), space="PSUM"))
            ptr_pool = _mstack.enter_context(
                tc.tile_pool(name="ptr", bufs=int(os.environ.get("DD_PTR", "1")), space="PSUM"))
            pchain = _mstack.enter_context(
                tc.tile_pool(name="pchain", bufs=int(os.environ.get("DD_PCH", "2")), space="PSUM"))

            act_prev = [None]
            nopin = bool(int(os.environ.get("DD_NOPIN", "0")))

            def act_chain(bi):
                if act_prev[0] is not None and not nopin:
                    add_dep_helper(bi.ins, act_prev[0].ins, sync=True,
                                   reason="pin ACT order for act-table reuse")
                act_prev[0] = bi
                return bi

            prev_row = [None, None, None]   # r_loc, l3row, r3

            def score_pair(t0, ptrt):
                # two consecutive tiles share one ee/attn buffer so the
                # softmax reduce / reciprocal / normalize run as single ops
                # (and one PSUM bank holds both tiles' scores)
                ee = score.tile([128, 2, 8, 32], f32, tag="ee", name="ee")
                for ti in range(2):
                    t = t0 + ti
                    r_loc, jt = TILES[t]
                    if prev_row[0] != r_loc:
                        prev_row[0] = r_loc
                        # lhsT rows: [ones; mu_p(row); invrstd(row)];
                        # rhs rows: [SA_i; -Sw; Sb]
                        l3row = l3_bufs[r_loc % 2]
                        nc.sync.dma_start(out=l3row[1:3, :],
                                          in_=stats_dram[:, r_loc, :])
                        r3 = prow.tile([3, 256], f32r, tag="r3", name="r3")
                        nc.sync.dma_start(out=r3, in_=r3_all[r_loc])
                        prev_row[1], prev_row[2] = l3row, r3
                    l3row, r3 = prev_row[1], prev_row[2]
                    jseg = slice(jt * 128, (jt + 1) * 128)
                    # scores-pre-rstd accumulated fully on PE:
                    #   px3 = SA_i - mu*Sw + invr*Sb  (rank-3)  +  I @ SA_j
                    px3 = px3_pool.tile([128, 256], f32, tag="px3", name="px3")
                    nc.tensor.matmul(px3, l3row[:, jseg], r3,
                                     start=True, stop=False)
                    nc.tensor.matmul(px3, ident, sa[:, jt, :],
                                     start=False, stop=True)
                    # E = exp(rstd * scores)
                    act_chain(nc.scalar.activation(
                        out=ee[:, ti, :, :].rearrange("p h t -> p (h t)"),
                        in_=px3,
                        func=AF.Exp,
                        scale=rstd_T[:, jt, r_loc:r_loc + 1]))
                den = small.tile([128, 2, 8], f32, tag="den", name="den")
                nc.vector.reduce_sum(out=den, in_=ee, axis=AX.X)
                rden = small.tile([128, 2, 8], f32, tag="rden", name="rden")
                nc.vector.reciprocal(out=rden, in_=den)
                attn = score.tile([128, 2, 8, 32], f32r, tag="attn", name="attn")
                nc.gpsimd.tensor_mul(attn, ee,
                                     rden.to_broadcast([128, 2, 8, 32]))
                for ti in range(2):
                    s = (t0 + ti) % CHUNK
                    a2 = attn[:, ti, :, :].rearrange("p h t -> p (h t)")
                    sseg = slice(s * 128, (s + 1) * 128)
                    nc.tensor.transpose(ptrt[:, 0, sseg], a2[:, 0:128], ident)
                    nc.tensor.transpose(ptrt[:, 1, sseg], a2[:, 128:256], ident)

            def chain_pair(aT_pair, q_pair):
                # two chunks share each silu: psum [128, 2, 512] spans two
                # banks, one [128, 1024] activation per (layer, mt) halves
                # the scalar engine's fixed per-op cost.
                def layer(x_of, w, b_tile, out_tile):
                    for mt in range(2):
                        ps = pchain.tile([128, 2, 512], f32, tag="pch",
                                         name="pch")
                        for qi in range(2):
                            for kt in range(2):
                                nc.tensor.matmul(
                                    ps[:, qi, :],
                                    w[:, kt, mt * 128:(mt + 1) * 128],
                                    x_of(qi, kt),
                                    start=(kt == 0), stop=(kt == 1))
                        act_chain(nc.scalar.activation(
                            out=out_tile[:, mt, :, :].rearrange(
                                "p q n -> p (q n)"),
                            in_=ps.rearrange("p q n -> p (q n)"), func=AF.Silu,
                            bias=b_tile[:, mt:mt + 1]))

                x2 = chainx.tile([128, 2, 2, 512], f32r, tag="x2", name="x2")
                layer(lambda qi, kt: aT_pair[qi][:, kt, :], w_a, b_a, x2)
                x4 = chainx.tile([128, 2, 2, 512], f32r, tag="x4", name="x4")
                layer(lambda qi, kt: x2[:, kt, qi, :], w_b, b_b, x4)
                x5 = chainx.tile([128, 2, 2, 512], f32r, tag="x5", name="x5")
                layer(lambda qi, kt: x4[:, kt, qi, :], w_d1, b_d1, x5)
                ps6 = pchain.tile([2, 2, 512], f32, tag="pch", name="ps6")
                for qi in range(2):
                    for kt in range(2):
                        nc.tensor.matmul(ps6[:, qi, :], w_d2[:, kt, :],
                                         x5[:, kt, qi, :],
                                         start=(kt == 0), stop=(kt == 1))
                # bias bd2 is added host-side during assembly
                o6 = small.tile([2, 2, 512], f32, tag="o6", name="o6")
                nc.vector.tensor_copy(out=o6, in_=ps6)
                for qi in range(2):
                    nc.sync.dma_start(out=out_ext[q_pair[qi]], in_=o6[:, qi, :])

            n_super = (NCHUNKS + GROUP - 1) // GROUP

            def phase_A(sc):
                qs = list(range(sc * GROUP, min((sc + 1) * GROUP, NCHUNKS)))
                aTs = []
                for q in qs:
                    ptrt = ptr_pool.tile([128, 2, 512], f32r, tag="ptrt",
                                         name="ptrt")
                    for s in range(0, CHUNK, 2):
                        score_pair(q * CHUNK + s, ptrt)
                    aT = attnT_pool.tile([128, 2, 512], f32r, tag="aT",
                                         name="aT")
                    nc.vector.tensor_copy(out=aT, in_=ptrt)
                    aTs.append(aT)
                return qs, aTs

            # chains run one superchunk behind scores, so the pinned ACT
            # order [exps(s)][exps(s+1)][silus(s)]... never stalls the
            # score pipeline on chain completion.
            def run_chains(p):
                qs, aTs = p
                for i in range(0, len(qs) - 1, 2):
                    chain_pair(aTs[i:i + 2], qs[i:i + 2])

            pending = None
            for sc in range(n_super):
                qa = phase_A(sc)
                if pending:
                    run_chains(pending)
                pending = qa
            if pending:
                run_chains(pending)
            _mstack.close()
    nc.compile()
    return nc


def _get_nc(st):
    key = ("nc", st[0], st[1], tuple(st[4]), st[5])
    if key not in _CACHE:
        _CACHE[key] = _build_nc(st)
    return _CACHE[key]


def kernel(**inputs):
    from concourse.bass_utils import run_bass_kernel_spmd

    pc = _precompute(inputs)
    U = pc["U"]
    st = _structure(U)
    NROWSC, NBLKJ, vmap, core_rows, TILES, NCHUNKS, ntiles_real = st
    in_maps = [_core_inputs(pc, st, c) for c in range(NCORES)]
    nc = _get_nc(st)
    res = run_bass_kernel_spmd(nc, in_maps, core_ids=list(range(NCORES)),
                               trace=bool(int(os.environ.get("DD_TRACE", "0"))))
    _CACHE["last_result"] = res

    Rc = np.zeros((U, U, 2), np.float32)
    filled = np.zeros((U, U), bool)
    for c in range(NCORES):
        o = res.results[c]["out"] + pc["bd2"][None, :, None]   # (NCHUNKS, 2, 512)
        ot = o.reshape(NCHUNKS, 2, CHUNK, 128).transpose(0, 2, 1, 3).reshape(-1, 2, 128)
        for t in range(NCHUNKS * CHUNK):
            r_loc, jt = TILES[t]
            a = core_rows[c][r_loc]
            cols = vmap[jt * 128:(jt + 1) * 128]
            Rc[a, cols, 0] = ot[t, 0]
            Rc[a, cols, 1] = ot[t, 1]
            filled[a, cols] = True
    missing = ~filled
    Rc[missing] = Rc.transpose(1, 0, 2)[missing]

    rho_c = (Rc[:, :, 0] + 1j * Rc[:, :, 1]).astype(np.complex64)
    inverse = pc["inverse"]
    rho = rho_c[np.ix_(inverse, inverse)]
    n_spin = int(np.asarray(inputs["n_spin"]))
    return np.broadcast_to(rho[None], (n_spin, NB, NB)).copy()


# revision 14
# speedup vs baseline: 1.7069x; 1.0162x over previous
"""Trainium2 Bass kernel for nn_DensityDecoder (gnn_message_passing).

Math: the reference computes, for every ordered pair (i, j) of NB=640 orbitals,
    pair = orb_i + orb_j                       (orb: per-orbital projected embedding)
    qn   = LayerNorm(pair) ; q = qn @ Wq + bq
    attn = softmax(q . k / sqrt(Dh)) over a tiny T=32 latent KV
    out  = MLP(attn @ V @ Wo)  ->  2 values -> rho[i, j] = out0 + 1j*out1

Because pair = orb_i + orb_j, the LN statistics decompose exactly:
    mu_ij  = mu_i + mu_j
    var_ij = msq_i + msq_j + 2*G_ij - mu_ij^2        (G = orb @ orb.T / D)
and the whole pre-softmax pipeline collapses to per-orbital precomputes
(SA = ((orb*g) @ Wq) projected into (head, token) score space, plus the
constant vectors Sw, Sb), so the 410MB pair tensor is never materialized:
    scores_ij = rstd_ij * (SA_i + SA_j - mu_ij*Sw) + Sb      (pre-scaled by 1/sqrt(Dh))
The per-pair device work is the softmax + a 5-layer MLP chain, where
attn @ V @ Wo is folded into one 256x256 matmul (Wvo = blockdiag(V) @ Wo).

Dedup: orb_i depends only on the (Z, l, m) triple, so rows with equal triples
are identical.  The device computes rho over the U distinct classes only
(U=396 for the reference inputs vs NB=640), and the host scatters
rho[i, j] = rho_class[cls[i], cls[j]].  rho_class is symmetric, so only
class pairs a <= b are computed; j is tiled into 128-wide virtual blocks
[0,128), [128,256), ..., [U-128, U) (last block right-aligned so padding
never exceeds one block), and class-rows are grouped by the j-blocks they
need: rows in [128g, 128(g+1)) take blocks g..NBLKJ-1, rows in the last
block take only the final one.

Sharding: class-rows are striped across the 8 cores (slot % 8 == core) within
each group, giving every core an IDENTICAL instruction stream (same NEFF,
SPMD) over different data.
"""

import os
import numpy as np
import ml_dtypes

EPS = 1e-5
H = 8
D = 256
T = 32
NB = 640
NCORES = 8
CHUNK = 4                 # tiles per MLP-chain chunk (512 pair columns)
GROUP = int(os.environ.get("DD_GROUP", "8"))  # chunks per superchunk

_CACHE = {}


def _silu(x):
    return x / (1.0 + np.exp(-x))


def _ln(x, g, b):
    mu = x.mean(-1, keepdims=True)
    var = x.var(-1, keepdims=True)
    return (x - mu) / np.sqrt(var + EPS) * g + b


def _structure(U):
    """Tile structure over U distinct classes.

    Returns (NROWSC, NBLKJ, vmap, core_rows, TILES, NCHUNKS, ntiles_real):
      vmap[v]       class index of virtual j column v (NBLKJ*128 columns)
      core_rows[c]  class index of each row slot on core c (NROWSC slots)
      TILES         [(row_slot, jt)] identical on every core
    """
    Upad = max(U, 128)
    NBLKJ = -(-Upad // 128)
    assert NBLKJ <= 8, "stats psum segment would exceed a PSUM bank"
    starts = [128 * k for k in range(NBLKJ - 1)] + [Upad - 128]
    bounds = starts + [Upad]
    vmap = np.concatenate(
        [np.clip(np.arange(s, s + 128), 0, U - 1) for s in starts])

    core_rows = [[] for _ in range(NCORES)]
    tiles = []
    for g in range(NBLKJ):
        rows = [min(r, U - 1) for r in range(bounds[g], bounds[g + 1])]
        while len(rows) % NCORES:
            rows.append(rows[0])
        nloc = len(rows) // NCORES
        base = len(core_rows[0])
        for c in range(NCORES):
            core_rows[c].extend(rows[c::NCORES])
        for sl in range(nloc):
            for jt in range(g, NBLKJ):
                tiles.append((base + sl, jt))
    ntiles_real = len(tiles)
    while len(tiles) % (2 * CHUNK):
        tiles.append(tiles[-1])
    NROWSC = len(core_rows[0])
    NCHUNKS = len(tiles) // CHUNK
    return NROWSC, NBLKJ, vmap, core_rows, tiles, NCHUNKS, ntiles_real


def _precompute(inputs):
    """Class-level precompute (all O(U*D) or smaller), numpy float64 -> float32."""
    f = {}
    for k, v in inputs.items():
        v = np.asarray(v)
        f[k] = v.astype(np.float64) if v.dtype in (np.float32, np.float64) else v
    Z = np.asarray(inputs["Z"]).astype(np.int64)
    l = np.asarray(inputs["l"]).astype(np.int64)
    m = np.asarray(inputs["m"]).astype(np.int64)
    m_idx = np.clip(m + 3, 0, 4)
    key = np.stack([Z, l, m_idx], 1)
    uniq, inverse = np.unique(key, axis=0, return_inverse=True)
    U = len(uniq)

    emb = np.concatenate(
        [f["elem_tab"][uniq[:, 0]], f["l_tab"][uniq[:, 1]], f["m_tab"][uniq[:, 2]]], -1)
    orb = _silu(emb @ f["Wp0"] + f["bp0"]) @ f["Wp1"] + f["bp1"]          # (U, D)

    kv = _ln(f["latent"], f["ln_gkv"], f["ln_bkv"])
    k = (kv @ f["Wk"] + f["bk"]).reshape(T, H, D // H)
    v = (kv @ f["Wv"] + f["bv"]).reshape(T, H, D // H)

    g, b = f["ln_gq"], f["ln_bq"]
    mu = orb.mean(-1)
    msq = (orb * orb).mean(-1)

    A = (orb * g) @ f["Wq"]
    wbar = g @ f["Wq"]
    bq_eff = b @ f["Wq"] + f["bqa"]

    kT = k.transpose(1, 2, 0)                                            # (H, Dh, T)
    scale = 1.0 / np.sqrt(np.float64(D // H))

    def to_scores(x):
        xh = x.reshape(x.shape[:-1] + (H, D // H))
        return (np.einsum('...hd,hdt->...ht', xh, kT).reshape(x.shape[:-1] + (H * T,))
                * scale)

    SA = to_scores(A)                                                    # (U, 256)
    Sw = to_scores(wbar)                                                 # (256,)
    Sb = to_scores(bq_eff)                                               # (256,)
    Wvo = np.einsum('thd,hde->hte', v, f["Wo"].reshape(H, D // H, D)).reshape(H * T, D)
    # fuse consecutive linear layers (no nonlinearity between them):
    # y2 = silu(attn @ Wa + ba); y4 = silu(y2 @ Wb + bb); y5 = silu(y4 @ Wd1 + bd1)
    Wa = Wvo @ f["Wt0"]
    ba = f["bo"] @ f["Wt0"] + f["bt0"]
    Wb = f["Wt1"] @ f["Wd0"]
    bb = f["bt1"] @ f["Wd0"] + f["bd0"]

    fl = lambda x: np.ascontiguousarray(x, np.float32)
    return {
        "U": U, "inverse": inverse,
        "SA": fl(SA), "Sw": fl(Sw), "Sb": fl(Sb), "mu": fl(mu), "msq": fl(msq),
        "orbT_s": fl(orb.T * np.sqrt(2.0 / D)),                          # (D, U)
        "Wa": fl(Wa), "ba": fl(ba), "Wb": fl(Wb), "bb": fl(bb),
        "Wd1": fl(f["Wd1"]), "bd1": fl(f["bd1"]),
        "Wd2": fl(f["Wd2"]), "bd2": fl(f["bd2"]),
    }


def _core_inputs(pc, st, c):
    NROWSC, NBLKJ, vmap, core_rows, tiles, NCHUNKS, _ = st
    rows = core_rows[c]
    ones_r = np.ones(NROWSC, np.float32)
    ones_v = np.ones(len(vmap), np.float32)
    return {
        "sa_in": np.ascontiguousarray(pc["SA"][vmap]),
        "orbT_in": np.ascontiguousarray(pc["orbT_s"][:, vmap]),
        "orbTc_in": np.ascontiguousarray(pc["orbT_s"][:, rows]),
        "lhs_mu": np.ascontiguousarray(np.stack([ones_r, pc["mu"][rows]])),
        "lhs_msq": np.ascontiguousarray(np.stack([ones_r, pc["msq"][rows]])),
        "rhs_mu": np.ascontiguousarray(np.stack([pc["mu"][vmap], ones_v])),
        "rhs_msq": np.ascontiguousarray(np.stack([pc["msq"][vmap], ones_v])),
        "r3_all": np.ascontiguousarray(np.stack(
            [np.stack([pc["SA"][i], -pc["Sw"], pc["Sb"]]) for i in rows])),
        "ident_in": np.eye(128, dtype=np.float32),
        "identb_in": np.eye(128, dtype=np.float32).astype(ml_dtypes.bfloat16),
        "ones_in": np.ones((1, NBLKJ * 128), np.float32),
        "wa": pc["Wa"].astype(ml_dtypes.bfloat16),
        "wb": pc["Wb"], "wd1": pc["Wd1"], "wd2": pc["Wd2"],
        "ba_in": pc["ba"], "bb_in": pc["bb"],
        "bd1_in": pc["bd1"], "bd2_in": pc["bd2"],
    }


def _build_nc(st):
    NROWSC, NBLKJ, _vmap, _core_rows, TILES, NCHUNKS, _ = st
    NBV = NBLKJ * 128
    import concourse.bass as bass
    import concourse.bacc as bacc
    import concourse.tile as tile
    from concourse import mybir
    dt = mybir.dt
    f32 = dt.float32
    f32r = dt.float32r
    AF = mybir.ActivationFunctionType
    AX = mybir.AxisListType

    nc = bacc.Bacc(None, target_bir_lowering=False)

    ein = lambda name, shape, d=f32: nc.dram_tensor(name, shape, d,
                                                     kind="ExternalInput")
    bf16_in = dt.bfloat16
    sa_in = ein("sa_in", [NBV, 256], f32r)
    orbT_in = ein("orbT_in", [D, NBV], f32r)
    orbTc_in = ein("orbTc_in", [D, NROWSC], f32r)
    lhs_mu = ein("lhs_mu", [2, NROWSC], f32r)
    lhs_msq = ein("lhs_msq", [2, NROWSC], f32r)
    rhs_mu = ein("rhs_mu", [2, NBV], f32r)
    rhs_msq = ein("rhs_msq", [2, NBV], f32r)
    r3_all = ein("r3_all", [NROWSC, 3, 256], f32r)
    ident_in = ein("ident_in", [128, 128], f32r)
    identb_in = ein("identb_in", [128, 128], bf16_in)
    ones_in = ein("ones_in", [1, NBV], f32r)
    wa = ein("wa", [256, 256], bf16_in)
    wb = ein("wb", [256, 256], f32r)
    wd1 = ein("wd1", [256, 256], f32r)
    wd2 = ein("wd2", [256, 2], f32r)
    ba_in = ein("ba_in", [256])
    bb_in = ein("bb_in", [256])
    bd1_in = ein("bd1_in", [256])
    bd2_in = ein("bd2_in", [2])

    out_ext = nc.dram_tensor("out", [NCHUNKS, 2, 512], f32, kind="ExternalOutput")
    stats_dram = nc.dram_tensor("stats_scratch", [2, NROWSC, NBV], f32r)

    bf16 = dt.bfloat16
    with tile.TileContext(nc) as tc, \
            nc.allow_low_precision(reason="fp32r matmul pipeline by design"):
        with (
            tc.tile_pool(name="const", bufs=1) as const,
            tc.tile_pool(name="prow", bufs=2) as prow,
            tc.tile_pool(name="score", bufs=int(os.environ.get("DD_SCORE", "5"))) as score,
            tc.tile_pool(name="small", bufs=5) as small,
            tc.tile_pool(name="attnT", bufs=2 * GROUP + 3) as attnT_pool,
            tc.tile_pool(name="chainx", bufs=int(os.environ.get("DD_CHX", "2"))) as chainx,
        ):
            # ---- constants into SBUF (prologue inputs first, so the stats
            # pipeline starts before the big weight loads finish) ----
            lmu = const.tile([2, NROWSC], f32r)
            nc.sync.dma_start(out=lmu, in_=lhs_mu[:])
            lmsq = const.tile([2, NROWSC], f32r)
            nc.sync.dma_start(out=lmsq, in_=lhs_msq[:])
            rmu = const.tile([2, NBV], f32r)
            nc.sync.dma_start(out=rmu, in_=rhs_mu[:])
            rmsq = const.tile([2, NBV], f32r)
            nc.sync.dma_start(out=rmsq, in_=rhs_msq[:])
            orbTc = const.tile([128, 2, NROWSC], f32r)
            nc.sync.dma_start(out=orbTc, in_=orbTc_in.rearrange("(k p) m -> p k m", p=128))
            orbT = const.tile([128, 2, NBV], f32r)
            nc.sync.dma_start(out=orbT, in_=orbT_in.rearrange("(k p) n -> p k n", p=128))
            sa = const.tile([128, NBLKJ, 256], f32r)
            nc.sync.dma_start(out=sa, in_=sa_in.rearrange("(jt p) c -> p jt c", p=128))

            w_a = const.tile([128, 2, 256], bf16)
            nc.sync.dma_start(out=w_a, in_=wa.rearrange("(k p) n -> p k n", p=128))
            w_b = const.tile([128, 2, 256], f32r)
            nc.sync.dma_start(out=w_b, in_=wb.rearrange("(k p) n -> p k n", p=128))
            w_d1 = const.tile([128, 2, 256], f32r)
            nc.sync.dma_start(out=w_d1, in_=wd1.rearrange("(k p) n -> p k n", p=128))
            w_d2 = const.tile([128, 2, 2], f32r)
            nc.sync.dma_start(out=w_d2, in_=wd2.rearrange("(k p) n -> p k n", p=128))

            b_a = const.tile([128, 2], f32)
            nc.sync.dma_start(out=b_a, in_=ba_in.rearrange("(m p) -> p m", p=128))
            b_b = const.tile([128, 2], f32)
            nc.sync.dma_start(out=b_b, in_=bb_in.rearrange("(m p) -> p m", p=128))
            b_d1 = const.tile([128, 2], f32)
            nc.sync.dma_start(out=b_d1, in_=bd1_in.rearrange("(m p) -> p m", p=128))
            b_d2 = const.tile([2, 1], f32)
            nc.sync.dma_start(out=b_d2, in_=bd2_in.rearrange("(p one) -> p one", one=1))

            ident = const.tile([128, 128], f32r)
            nc.sync.dma_start(out=ident, in_=ident_in[:])
            identb = const.tile([128, 128], bf16)
            nc.sync.dma_start(out=identb, in_=identb_in[:])
            l3_bufs = [const.tile([3, NBV], f32r, tag=f"l3_{i}", name=f"l3_{i}")
                       for i in range(2)]
            for lb in l3_bufs:
                nc.sync.dma_start(out=lb[0:1, :], in_=ones_in[:])
            eps_t = const.tile([NROWSC, 1], f32)
            nc.gpsimd.memset(eps_t, EPS)

            # persistent per-row stats
            mu_p_sb = const.tile([NROWSC, NBV], f32r)
            invr_sb = const.tile([NROWSC, NBV], f32r)
            rstd_sb = const.tile([NROWSC, NBV], f32r)
            rstd_T = const.tile([128, NBLKJ, NROWSC], f32)

            # ---- prologue: per-pair LN stats for this core's rows ----
            with (
                tc.tile_pool(name="pro_ps", bufs=2, space="PSUM") as pro_ps,
                tc.tile_pool(name="pro_sb", bufs=2) as pro_sb,
            ):
                seg_w = NBV // 2
                for nch in range(2):
                    seg = slice(nch * seg_w, (nch + 1) * seg_w)
                    psA = pro_ps.tile([NROWSC, seg_w], f32, tag="psA")
                    nc.tensor.matmul(psA, lmu, rmu[:, seg], start=True, stop=True)
                    nc.vector.tensor_copy(out=mu_p_sb[:, seg], in_=psA)
                    psB = pro_ps.tile([NROWSC, seg_w], f32, tag="psB")
                    nc.tensor.matmul(psB, lmsq, rmsq[:, seg], start=True, stop=False)
                    nc.tensor.matmul(psB, orbTc[:, 0, :], orbT[:, 0, seg],
                                     start=False, stop=False)
                    nc.tensor.matmul(psB, orbTc[:, 1, :], orbT[:, 1, seg],
                                     start=False, stop=True)
                    mu2 = pro_sb.tile([NROWSC, seg_w], f32, tag="mu2")
                    nc.vector.tensor_mul(mu2, mu_p_sb[:, seg], mu_p_sb[:, seg])
                    nc.vector.tensor_sub(invr_sb[:, seg], psB, mu2)
                # invr = sqrt(var + eps); rstd = 1/invr
                nc.scalar.activation(out=invr_sb, in_=invr_sb, func=AF.Sqrt,
                                     bias=eps_t[:, 0:1])
                nc.vector.reciprocal(out=rstd_sb, in_=invr_sb)
                nc.sync.dma_start(out=stats_dram[0], in_=mu_p_sb)
                nc.sync.dma_start(out=stats_dram[1], in_=invr_sb)
                for jt in range(NBLKJ):
                    pT = pro_ps.tile([128, NROWSC], f32r, tag="pT")
                    nc.tensor.transpose(
                        pT, rstd_sb[:, jt * 128:(jt + 1) * 128],
                        ident[0:NROWSC, 0:NROWSC])
                    nc.vector.tensor_copy(out=rstd_T[:, jt, :], in_=pT)

            # ---- main loop: superchunks separate Exp (phase A) from Silu
            # (phase B) on the scalar engine. Each activation-function switch
            # costs a ~1.3us InstLoadActFuncSet table load, so ACT program
            # order is pinned with order-only deps: [A exps][B silus] per
            # superchunk -> 2 table loads per superchunk instead of per tile.
            # Other engines still overlap phase B(s) with phase A(s+1).
            from concourse.tile_rust import add_dep_helper
            import contextlib
            _mstack = contextlib.ExitStack()
            px3_pool = _mstack.enter_context(
                tc.tile_pool(name="px3", bufs=int(os.environ.get("DD_PX3", "2")), space="PSUM"))
            ptr_pool = _mstack.enter_context(
                tc.tile_pool(name="ptr", bufs=int(os.environ.get("DD_PTR", "2")), space="PSUM"))
            pchain = _mstack.enter_context(
                tc.tile_pool(name="pchain", bufs=int(os.environ.get("DD_PCH", "2")), space="PSUM"))

            act_prev = [None]
            nopin = bool(int(os.environ.get("DD_NOPIN", "0")))

            def act_chain(bi):
                if act_prev[0] is not None and not nopin:
                    add_dep_helper(bi.ins, act_prev[0].ins, sync=True,
                                   reason="pin ACT order for act-table reuse")
                act_prev[0] = bi
                return bi

            prev_row = [None, None, None]   # r_loc, l3row, r3

            def score_pair(t0, ptrt):
                # two consecutive tiles share one ee/attn buffer so the
                # softmax reduce / reciprocal / normalize run as single ops
                # (and one PSUM bank holds both tiles' scores)
                ee = score.tile([128, 2, 8, 32], f32, tag="ee", name="ee")
                for ti in range(2):
                    t = t0 + ti
                    r_loc, jt = TILES[t]
                    if prev_row[0] != r_loc:
                        prev_row[0] = r_loc
                        # lhsT rows: [ones; mu_p(row); invrstd(row)];
                        # rhs rows: [SA_i; -Sw; Sb]
                        l3row = l3_bufs[r_loc % 2]
                        nc.sync.dma_start(out=l3row[1:3, :],
                                          in_=stats_dram[:, r_loc, :])
                        r3 = prow.tile([3, 256], f32r, tag="r3", name="r3")
                        nc.sync.dma_start(out=r3, in_=r3_all[r_loc])
                        prev_row[1], prev_row[2] = l3row, r3
                    l3row, r3 = prev_row[1], prev_row[2]
                    jseg = slice(jt * 128, (jt + 1) * 128)
                    # scores-pre-rstd accumulated fully on PE:
                    #   px3 = SA_i - mu*Sw + invr*Sb  (rank-3)  +  I @ SA_j
                    px3 = px3_pool.tile([128, 256], f32, tag="px3", name="px3")
                    nc.tensor.matmul(px3, l3row[:, jseg], r3,
                                     start=True, stop=False)
                    nc.tensor.matmul(px3, ident, sa[:, jt, :],
                                     start=False, stop=True)
                    # E = exp(rstd * scores)
                    act_chain(nc.scalar.activation(
                        out=ee[:, ti, :, :].rearrange("p h t -> p (h t)"),
                        in_=px3,
                        func=AF.Exp,
                        scale=rstd_T[:, jt, r_loc:r_loc + 1]))
                den = small.tile([128, 2, 8], f32, tag="den", name="den")
                nc.vector.reduce_sum(out=den, in_=ee, axis=AX.X)
                rden = small.tile([128, 2, 8], f32, tag="rden", name="rden")
                nc.vector.reciprocal(out=rden, in_=den)
                attn = score.tile([128, 2, 8, 32], bf16, tag="attn", name="attn")
                nc.gpsimd.tensor_mul(attn, ee,
                                     rden.to_broadcast([128, 2, 8, 32]))
                for ti in range(2):
                    s = (t0 + ti) % CHUNK
                    a2 = attn[:, ti, :, :].rearrange("p h t -> p (h t)")
                    sseg = slice(s * 128, (s + 1) * 128)
                    nc.tensor.transpose(ptrt[:, 0, sseg], a2[:, 0:128], identb)
                    nc.tensor.transpose(ptrt[:, 1, sseg], a2[:, 128:256], identb)

            def chain_pair(aT_pair, q_pair):
                # two chunks share each silu: psum [128, 2, 512] spans two
                # banks, one [128, 1024] activation per (layer, mt) halves
                # the scalar engine's fixed per-op cost.
                def layer(x_of, w, b_tile, out_tile):
                    for mt in range(2):
                        ps = pchain.tile([128, 2, 512], f32, tag="pch",
                                         name="pch")
                        for qi in range(2):
                            for kt in range(2):
                                nc.tensor.matmul(
                                    ps[:, qi, :],
                                    w[:, kt, mt * 128:(mt + 1) * 128],
                                    x_of(qi, kt),
                                    start=(kt == 0), stop=(kt == 1))
                        act_chain(nc.scalar.activation(
                            out=out_tile[:, mt, :, :].rearrange(
                                "p q n -> p (q n)"),
                            in_=ps.rearrange("p q n -> p (q n)"), func=AF.Silu,
                            bias=b_tile[:, mt:mt + 1]))

                x2 = chainx.tile([128, 2, 2, 512], f32r, tag="x2", name="x2")
                layer(lambda qi, kt: aT_pair[qi][:, kt, :], w_a, b_a, x2)
                x4 = chainx.tile([128, 2, 2, 512], f32r, tag="x4", name="x4")
                layer(lambda qi, kt: x2[:, kt, qi, :], w_b, b_b, x4)
                x5 = chainx.tile([128, 2, 2, 512], f32r, tag="x5", name="x5")
                layer(lambda qi, kt: x4[:, kt, qi, :], w_d1, b_d1, x5)
                ps6 = pchain.tile([2, 2, 512], f32, tag="pch", name="ps6")
                for qi in range(2):
                    for kt in range(2):
                        nc.tensor.matmul(ps6[:, qi, :], w_d2[:, kt, :],
                                         x5[:, kt, qi, :],
                                         start=(kt == 0), stop=(kt == 1))
                # bias bd2 is added host-side during assembly
                o6 = small.tile([2, 2, 512], f32, tag="o6", name="o6")
                nc.vector.tensor_copy(out=o6, in_=ps6)
                for qi in range(2):
                    nc.sync.dma_start(out=out_ext[q_pair[qi]], in_=o6[:, qi, :])

            n_super = (NCHUNKS + GROUP - 1) // GROUP

            def phase_A(sc):
                qs = list(range(sc * GROUP, min((sc + 1) * GROUP, NCHUNKS)))
                aTs = []
                for q in qs:
                    ptrt = ptr_pool.tile([128, 2, 512], bf16, tag="ptrt",
                                         name="ptrt")
                    for s in range(0, CHUNK, 2):
                        score_pair(q * CHUNK + s, ptrt)
                    aT = attnT_pool.tile([128, 2, 512], bf16, tag="aT",
                                         name="aT")
                    nc.vector.tensor_copy(out=aT, in_=ptrt)
                    aTs.append(aT)
                return qs, aTs

            # chains run one superchunk behind scores, so the pinned ACT
            # order [exps(s)][exps(s+1)][silus(s)]... never stalls the
            # score pipeline on chain completion.
            def run_chains(p):
                qs, aTs = p
                for i in range(0, len(qs) - 1, 2):
                    chain_pair(aTs[i:i + 2], qs[i:i + 2])

            pending = None
            for sc in range(n_super):
                qa = phase_A(sc)
                if pending:
                    run_chains(pending)
                pending = qa
            if pending:
                run_chains(pending)
            _mstack.close()
    nc.compile()
    return nc


def _get_nc(st):
    key = ("nc", st[0], st[1], tuple(st[4]), st[5])
    if key not in _CACHE:
        _CACHE[key] = _build_nc(st)
    return _CACHE[key]


def kernel(**inputs):
    from concourse.bass_utils import run_bass_kernel_spmd

    pc = _precompute(inputs)
    U = pc["U"]
    st = _structure(U)
    NROWSC, NBLKJ, vmap, core_rows, TILES, NCHUNKS, ntiles_real = st
    in_maps = [_core_inputs(pc, st, c) for c in range(NCORES)]
    nc = _get_nc(st)
    res = run_bass_kernel_spmd(nc, in_maps, core_ids=list(range(NCORES)),
                               trace=bool(int(os.environ.get("DD_TRACE", "0"))))
    _CACHE["last_result"] = res

    Rc = np.zeros((U, U, 2), np.float32)
    filled = np.zeros((U, U), bool)
    for c in range(NCORES):
        o = res.results[c]["out"] + pc["bd2"][None, :, None]   # (NCHUNKS, 2, 512)
        ot = o.reshape(NCHUNKS, 2, CHUNK, 128).transpose(0, 2, 1, 3).reshape(-1, 2, 128)
        for t in range(NCHUNKS * CHUNK):
            r_loc, jt = TILES[t]
            a = core_rows[c][r_loc]
            cols = vmap[jt * 128:(jt + 1) * 128]
            Rc[a, cols, 0] = ot[t, 0]
            Rc[a, cols, 1] = ot[t, 1]
            filled[a, cols] = True
    missing = ~filled
    Rc[missing] = Rc.transpose(1, 0, 2)[missing]

    rho_c = (Rc[:, :, 0] + 1j * Rc[:, :, 1]).astype(np.complex64)
    inverse = pc["inverse"]
    rho = rho_c[np.ix_(inverse, inverse)]
    n_spin = int(np.asarray(inputs["n_spin"]))
    return np.broadcast_to(rho[None], (n_spin, NB, NB)).copy()
